# revision 63
# baseline (speedup 1.0000x reference)
"""Masked multi-head self-attention block on 8 Trainium2 NeuronCores.

Strategy: pure data-parallel over batch (B=8 -> 1 batch per core, no
collectives). Per-core program is a transpose-free matmul chain:

  host feeds x^T [C,N], w_qk^T [C,2C] (q pre-scaled), w_v^T, w_proj^T,
  exp(mask^T), plus a bias-broadcast tile.

  M1a: qk^T[o,n]   = (w_qk^T).T-chain  (lhsT=w tile, rhs=x^T)      K=c
  M1b: v[n,o_v]    = (x^T).T @ w_v^T   (lhsT=x^T tile, rhs=w_v^T)  K=c
       v stored augmented [n, 16*65] with a ones column per head.
  M2 : s^T[m,n]    = k_h^T.T @ q_h^T  per head                     K=d=64
       The two heads of a pair run on row-groups (0-63 / 64-127) with
       interleaved issue so both 64-row strips stream CONCURRENTLY.
       p = exp(s^T) * exp(mask^T)      (ACT exp; mask-mul split
       between DVE and GPSIMD to balance engine load)
  M3 : outa^T[65,n] = v_aug.T @ p^T   accumulated over m-tiles     K=m
       rows 0..63 = out_h^T, row 64 = softmax denominator (ones col).
       Runs one pair behind M2 (one PSUM bank per chunk sequence).
  norm: one reciprocal_approx_fast per pair on the packed den row,
       bf16 rank-1 broadcast matmuls (rows 0-63 <- 1/den_h0, rows
       64-127 <- 1/den_h1), in-place muls; deferred two pairs so the
       reciprocal chain never stalls the PE at a pair boundary.
  M4 : y[n,o']     = (out^T).T @ w_proj^T + b                      K=c

Matmuls run in bf16. Softmax math stays f32: logits are PSUM-f32,
exp(f32)->bf16 attention weights, PE accumulation in f32 PSUM.
The wpsb tiles serve double duty: w_v^T during phase A, then reloaded
with w_proj^T for phase C (saves 16KB/partition of SBUF).
"""

import sys

sys.path.insert(0, "/opt/trn_rl_repo")

from contextlib import ExitStack

import numpy as np

import concourse.bass as bass
import concourse.tile as tile
from concourse import mybir

B, N, C, H, D = 8, 1024, 1024, 16, 64
SCALE = D**-0.5
F32 = mybir.dt.float32
BF16 = mybir.dt.bfloat16
NT = 8  # 128-row tiles over n (and m)
CT = 8  # 128-row tiles over c
NCH = 2  # 512-wide chunks over n
NP = H // 2  # head pairs
VW = H * 128  # per head: 64 v cols + 64 ones cols (full-M matmul, free denoms)


def _emit(ctx, tc):
    nc = tc.nc
    xT = nc.declare_dram_parameter("xT", [C, N], BF16, isOutput=False)
    expm = nc.declare_dram_parameter("expm", [N, N], BF16, isOutput=False)
    # per-pair qk weights, host-relaid so each pair is one contiguous
    # [128, 2048] block (4KB DMA lines): cols ct*256+j = (q_j | k_j-128)
    wqk2 = nc.declare_dram_parameter("wqk2", [NP * 128, 2 * C], BF16, isOutput=False)
    wvT = nc.declare_dram_parameter("wvT", [C, C], BF16, isOutput=False)
    wpT = nc.declare_dram_parameter("wpT", [C, C], BF16, isOutput=False)
    bb = nc.declare_dram_parameter("bb", [128, C], F32, isOutput=False)
    e2 = nc.declare_dram_parameter("e2", [2, 128], BF16, isOutput=False)
    y = nc.declare_dram_parameter("y", [N, C], F32, isOutput=True)

    Exp = mybir.ActivationFunctionType.Exp

    # ---- persistent SBUF ----
    per = ctx.enter_context(tc.tile_pool(name="per", bufs=1))
    vA = [per.tile([128, VW], BF16, tag=f"v{i}", name=f"vA{i}") for i in range(NT)]
    outT = [per.tile([128, N], BF16, tag=f"o{i}", name=f"outT{i}") for i in range(NT)]
    bb_sb = per.tile([128, C], F32, tag="bb")
    e2a_sb = per.tile([1, 128], BF16, tag="e2a")
    e2b_sb = per.tile([1, 128], BF16, tag="e2b")
    nc.sync.dma_start(e2a_sb[:], e2[0:1, :])
    nc.sync.dma_start(e2b_sb[:], e2[1:2, :])
    msb = [per.tile([128, N], BF16, tag=f"m{i}", name=f"msb{i}") for i in range(NT)]
    # wpsb: holds w_v^T during phase A, reloaded with w_proj^T for phase C
    wpsb = [per.tile([128, C], BF16, tag=f"wp{i}", name=f"wpsb{i}") for i in range(CT)]
    xsb = [per.tile([128, N], BF16, tag=f"x{i}", name=f"xsb{i}") for i in range(CT)]

    # pools shared across phases (PSUM: psW 2 + psS 4 + psO 2 = 8 banks)
    psW = ctx.enter_context(tc.tile_pool(name="psW", bufs=2, space="PSUM"))
    psS = ctx.enter_context(tc.tile_pool(name="psS", bufs=2, space="PSUM"))
    psO = ctx.enter_context(tc.tile_pool(name="psO", bufs=2, space="PSUM"))
    qkp = ctx.enter_context(tc.tile_pool(name="qk", bufs=2))
    pp = ctx.enter_context(tc.tile_pool(name="p", bufs=20))
    pe_p = ctx.enter_context(tc.tile_pool(name="pe", bufs=3))
    dnp = ctx.enter_context(tc.tile_pool(name="dn", bufs=1))
    wqp = ctx.enter_context(tc.tile_pool(name="wq", bufs=2))
    yp = ctx.enter_context(tc.tile_pool(name="y", bufs=3))

    # ---- DMA queue: phase A + prologue needs first (full-width loads
    # keep 2-4KB lines; spread across engine rings to stack DMA BW
    # while all engines are idle during the ramp) ----
    rings = [nc.sync, nc.gpsimd, nc.scalar]
    # first halves of x feed the pair-0 qk prologue ASAP
    for i in range(CT):
        rings[i % 3].dma_start(xsb[i][:, 0:512], xT[i * 128 : (i + 1) * 128, 0:512])

    def load_wts(hp, eng=None):
        w2 = wqp.tile([128, 2 * C], BF16, tag="wt", name="wt")
        (eng or nc.sync).dma_start(w2[:], wqk2[hp * 128 : (hp + 1) * 128, :])
        return w2

    w2_cur = load_wts(0, eng=nc.gpsimd)
    for i in range(CT):
        rings[i % 3].dma_start(
            wpsb[i][:, 0:512], wvT[i * 128 : (i + 1) * 128, 0:512]
        )
    for i in range(CT):
        rings[i % 3].dma_start(
            xsb[i][:, 512:1024], xT[i * 128 : (i + 1) * 128, 512:1024]
        )
    for i in range(CT):
        rings[i % 3].dma_start(
            wpsb[i][:, 512:1024], wvT[i * 128 : (i + 1) * 128, 512:1024]
        )
    nc.scalar.dma_start(bb_sb[:], bb[:])
    for i in range(NT):
        rings[i % 2].dma_start(msb[i][:], expm[i * 128 : (i + 1) * 128, :])

    # ones blocks of vA (interleaved [64 v | 64 ones] per head)
    clean1k = bb_sb[:, 0:1024].rearrange("p (h x) -> p h x", x=64)
    for mt in range(NT):
        ones_cols = vA[mt][:].rearrange("p (h x) -> p h x", x=128)[:, :, 64:128]
        nc.scalar.activation(
            ones_cols,
            clean1k,
            mybir.ActivationFunctionType.Copy,
            bias=1.0,
            scale=0.0,
        )

    def ptile(pool):
        tag = "w" if pool is psW else "ops"
        return pool.tile([128, 512], F32, tag=tag, name="ps5")

    def m1a_group(w2, qi, dst_qk, ns):
        """qi=0 for q, 1 for k; weights from the packed per-pair block."""
        ps = psW.tile([128, 512], F32, tag="w", name="psw")
        for ct in range(CT):
            nc.tensor.matmul(
                ps[:],
                w2[:, ct * 256 + qi * 128 : ct * 256 + (qi + 1) * 128],
                xsb[ct][:, ns],
                start=(ct == 0),
                stop=(ct == CT - 1),
            )
        nc.vector.tensor_copy(dst_qk[:, ns], ps[:])

    def phaseA_och(och):
        os_ = slice(och * 512, (och + 1) * 512)
        for mt in range(NT):
            ps = psW.tile([128, 512], F32, tag="w", name="psw")
            for ct in range(CT):
                nc.tensor.matmul(
                    ps[:],
                    xsb[ct][:, mt * 128 : (mt + 1) * 128],
                    wpsb[ct][:, os_],
                    start=(ct == 0),
                    stop=(ct == CT - 1),
                )
            dst = vA[mt][:, och * 8 * 128 : (och + 1) * 8 * 128]
            dst = dst.rearrange("p (h x) -> p h x", h=8)[:, :, 0:64]
            src = ps[:].rearrange("p (h d) -> p h d", h=8)
            nc.vector.tensor_copy(dst, src)

    # ---- phase A interleaved with pair-0 qk prologue ----
    qk_cur = (
        qkp.tile([128, N], BF16, tag="q", name="qk_q"),
        qkp.tile([128, N], BF16, tag="k", name="qk_k"),
    )
    m1a_group(w2_cur, 0, qk_cur[0], slice(0, 512))
    m1a_group(w2_cur, 1, qk_cur[1], slice(0, 512))
    phaseA_och(0)
    m1a_group(w2_cur, 0, qk_cur[0], slice(512, 1024))
    m1a_group(w2_cur, 1, qk_cur[1], slice(512, 1024))
    phaseA_och(1)
    # proj weights overwrite the w_v tiles (only needed in phase C)
    for i in range(CT):
        nc.sync.dma_start(wpsb[i][:], wpT[i * 128 : (i + 1) * 128, :])

    # ---- phase B ----
    def s_mms(qk_q, qk_k, mt, psa, psb):
        """Interleaved score MMs: row-group 0 (h0) and 64 (h1) issue
        back-to-back per chunk so both strips stream concurrently."""
        ms = slice(mt * 128, (mt + 1) * 128)
        for nch in range(NCH):
            ns = slice(nch * 512, (nch + 1) * 512)
            for row, ps in ((0, psa), (1, psb)):
                rp = slice(row * 64, row * 64 + 64)
                nc.tensor.matmul(
                    ps[:, ns],
                    qk_k[rp, ms],
                    qk_q[rp, ns],
                    start=True,
                    stop=True,
                    tile_position=(row * 64, 0),
                )

    def exp_mul(ps, mt, on_gpsimd):
        pe_t = pe_p.tile([128, N], BF16, name="pe_t")
        nc.scalar.activation(pe_t[:], ps[:], Exp)
        pt = pp.tile([128, N], BF16, name="pt")
        eng = nc.gpsimd if on_gpsimd else nc.vector
        eng.tensor_mul(pt[:], pe_t[:], msb[mt][:])
        return pt

    def make_m3_steps(hp, pts0, pts1, stg):
        """M3 for pair hp: per head, per chunk, 8 accumulating MMs into
        one PSUM bank, then evacuate (outT rows on DVE, den row packed
        into stg[0, h*N+ns] on ACT/DVE alternating)."""
        steps = []
        state = {}
        for hi, pts in ((0, pts0), (1, pts1)):
            h = 2 * hp + hi
            for nch in range(NCH):
                ns = slice(nch * 512, (nch + 1) * 512)
                for mt in range(NT):

                    def mm(h=h, ns=ns, mt=mt, nch=nch, pts=pts):
                        if mt == 0:
                            state[(h, nch)] = psO.tile([128, 512], F32, name="ops")
                        nc.tensor.matmul(
                            state[(h, nch)][:],
                            vA[mt][:, h * 128 : (h + 1) * 128],
                            pts[mt][:, ns],
                            start=(mt == 0),
                            stop=(mt == NT - 1),
                        )

                    steps.append(mm)

                def evac(h=h, hi=hi, nch=nch, ns=ns):
                    ops = state.pop((h, nch))
                    qp = hi * 64
                    nc.vector.tensor_copy(outT[hp][qp : qp + 64, ns], ops[0:64, :])
                    ds = slice(hi * N + nch * 512, hi * N + (nch + 1) * 512)
                    if nch == 0:
                        nc.scalar.copy(stg[0:1, ds], ops[64:65, :])
                    else:
                        nc.vector.tensor_copy(stg[0:1, ds], ops[64:65, :])

                steps.append(evac)
        return steps

    def norm_pair(hp, stg):
        """one packed reciprocal for both heads' denominators, broadcast
        across partitions via two accumulating rank-1 matmuls (rows 0-63
        get 1/den_h0, rows 64-127 get 1/den_h1), then in-place muls."""
        rec = dnp.tile([1, 2 * N], F32, tag="rec", name="rec")
        nc.vector.reciprocal_approx_fast(rec[:], stg[:])
        recb = dnp.tile([1, 2 * N], BF16, tag="recb", name="recb")
        nc.vector.tensor_copy(recb[:], rec[:])
        for nch in range(NCH):
            ns = slice(nch * 512, (nch + 1) * 512)
            bc = psW.tile([128, 512], F32, tag="w", name="psw")
            nc.tensor.matmul(
                bc[:], e2a_sb[:], recb[0:1, ns], start=True, stop=False,
                tile_position=(0, 0),
            )
            nc.tensor.matmul(
                bc[:], e2b_sb[:], recb[0:1, N + nch * 512 : N + (nch + 1) * 512],
                start=False, stop=True, tile_position=(0, 0),
            )
            nc.vector.tensor_mul(outT[hp][:, ns], outT[hp][:, ns], bc[:])

    def norm_head(hp, hi, stg):
        """normalize one head only (used for the last pair so phase C
        isn't gated behind the full reciprocal chain)."""
        rec = dnp.tile([1, N], F32, tag="rec", name="recH")
        nc.vector.reciprocal_approx_fast(rec[:], stg[0:1, hi * N : (hi + 1) * N])
        recb = dnp.tile([1, N], BF16, tag="recb", name="recbH")
        nc.vector.tensor_copy(recb[:], rec[:])
        esb = e2a_sb if hi == 0 else e2b_sb
        qp = hi * 64
        for nch in range(NCH):
            ns = slice(nch * 512, (nch + 1) * 512)
            bc = psW.tile([128, 512], F32, tag="w", name="psw")
            nc.tensor.matmul(
                bc[:], esb[:], recb[0:1, ns], start=True, stop=True,
                tile_position=(0, 0),
            )
            nc.vector.tensor_mul(
                outT[hp][qp : qp + 64, ns],
                outT[hp][qp : qp + 64, ns],
                bc[qp : qp + 64, :],
            )

    prev = None
    pending_norm = None  # (hp, stg): normalized two pairs behind, so the
    # reciprocal chain never blocks the PE at a pair boundary
    for hp in range(NP):
        qk_q, qk_k = qk_cur
        if hp + 1 < NP:
            w2_next = load_wts(hp + 1)
            qk_next = (
                qkp.tile([128, N], BF16, tag="q", name="qk_q"),
                qkp.tile([128, N], BF16, tag="k", name="qk_k"),
            )
            m1a_plan = [
                (w2_next, 0, qk_next[0], slice(0, 512)),
                (w2_next, 0, qk_next[0], slice(512, 1024)),
                (w2_next, 1, qk_next[1], slice(0, 512)),
                (w2_next, 1, qk_next[1], slice(512, 1024)),
            ]
        else:
            qk_next = None
            m1a_plan = []

        if prev is not None:
            p_hp, p_steps, p_stg = prev
            m3_iter = iter(p_steps)
        else:
            m3_iter = iter(())

        pts0, pts1 = [], []
        stg = dnp.tile([1, 2 * N], F32, tag="stg", bufs=3, name="stg")
        for mt in range(NT):
            psa = psS.tile([128, N], F32, tag="s", name="s0")
            psb = psS.tile([128, N], F32, tag="s", name="s1")
            s_mms(qk_q, qk_k, mt, psa, psb)
            # route 6/16 mask-muls to GPSIMD to unload the DVE
            pts0.append(exp_mul(psa, mt, on_gpsimd=(mt in (2, 6))))
            pts1.append(exp_mul(psb, mt, on_gpsimd=(mt % 2 == 1)))
            # drain previous pair's M3 work: 36 steps over 8 iterations
            for _ in range(5 if mt % 2 == 0 else 4):
                step = next(m3_iter, None)
                if step is not None:
                    step()
            if mt == 2 and pending_norm is not None:
                norm_pair(*pending_norm)
                pending_norm = None
            # last group at mt 6 (not 7) so the DVE evac of qk_k lands
            # a full iteration before the next pair's score MMs need it
            if mt in (1, 3, 5, 6) and m1a_plan:
                m1a_group(*m1a_plan[(1, 3, 5, 6).index(mt)])
        for step in m3_iter:
            step()
        if prev is not None:
            pending_norm = (p_hp, p_stg)
        prev = (hp, make_m3_steps(hp, pts0, pts1, stg), stg)
        qk_cur = qk_next

    # epilogue: last pair's M3 with per-head normalization interleaved
    # so phase C's start only waits on the second head's short chain
    p_hp, p_steps, p_stg = prev
    steps = iter(p_steps)
    for _ in range(18):
        next(steps)()
    if pending_norm is not None:
        norm_pair(*pending_norm)
        pending_norm = None
    norm_head(p_hp, 0, p_stg)
    for step in steps:
        step()
    norm_head(p_hp, 1, p_stg)

    # ---- phase C: projection (ct-inner pairs share the stationary
    # operand between both output chunks -> hidden weight loads; nt
    # sequences alternate PSUM pools so evacuation never stalls PE) ----
    for nt in range(NT):
        pss = [ptile(psW if nt % 2 == 0 else psO) for _ in range(NCH)]
        for ct in range(CT):
            lhsT = outT[ct][:, nt * 128 : (nt + 1) * 128]
            for och in range(NCH):
                nc.tensor.matmul(
                    pss[och][:],
                    lhsT,
                    wpsb[ct][:, och * 512 : (och + 1) * 512],
                    start=(ct == 0),
                    stop=(ct == CT - 1),
                )
        for och in range(NCH):
            os_ = slice(och * 512, (och + 1) * 512)
            yt = yp.tile([128, 512], F32)
            nc.vector.tensor_add(yt[:], pss[och][:], bb_sb[:, os_])
            ring = nc.sync if och == 0 else nc.gpsimd
            ring.dma_start(y[nt * 128 : (nt + 1) * 128, os_], yt[:])


def build_nc():
    from concourse import bacc

    nc = bacc.Bacc("TRN2", target_bir_lowering=False, debug=False)
    with tile.TileContext(nc) as tc, ExitStack() as ctx:
        _emit(ctx, tc)
    nc.compile()
    return nc


def host_prep(x, mask, w_qkv, w_proj, b_proj):
    """Per-core input maps (host-side layout prep only)."""
    x = np.asarray(x, np.float32)
    mask = np.asarray(mask, np.float32)
    w_qkv = np.asarray(w_qkv, np.float32)
    w_proj = np.asarray(w_proj, np.float32)
    b_proj = np.asarray(b_proj, np.float32)

    wq = w_qkv[0:C] * np.float32(SCALE)
    wk = w_qkv[C : 2 * C]
    wv = w_qkv[2 * C : 3 * C]
    import ml_dtypes

    bf16 = ml_dtypes.bfloat16
    wqT = np.ascontiguousarray(wq.T)  # [C, C]
    wkT = np.ascontiguousarray(wk.T)
    # pack per-pair qk weights contiguously: wqk2[hp*128+p, ct*256+j]
    #   j<128 -> wqT[ct*128+p, hp*128+j]; j>=128 -> wkT[..., j-128]
    wqk2 = np.zeros((NP * 128, 2 * C), np.float32)
    for hp in range(NP):
        for ct in range(CT):
            rows = slice(ct * 128, (ct + 1) * 128)
            cols = slice(hp * 128, (hp + 1) * 128)
            wqk2[hp * 128 : (hp + 1) * 128, ct * 256 : ct * 256 + 128] = wqT[rows, cols]
            wqk2[hp * 128 : (hp + 1) * 128, ct * 256 + 128 : ct * 256 + 256] = wkT[
                rows, cols
            ]
    wqk2 = wqk2.astype(bf16)
    wvT = np.ascontiguousarray(wv.T).astype(bf16)  # [C, C]
    bbn = np.tile(b_proj[None, :], (128, 1)).astype(np.float32)
    wpT16 = np.ascontiguousarray(w_proj.T).astype(bf16)
    e2n = np.zeros((2, 128), np.float32)
    e2n[0, 0:64] = 1.0
    e2n[1, 64:128] = 1.0
    e2n = e2n.astype(bf16)

    in_maps = []
    for b in range(B):
        in_maps.append(
            {
                "xT": np.ascontiguousarray(x[b].T).astype(bf16),
                "expm": np.exp(np.ascontiguousarray(mask[b, 0].T)).astype(bf16),
                "wqk2": wqk2,
                "wvT": wvT,
                "wpT": wpT16,
                "bb": bbn,
                "e2": e2n,
            }
        )
    return in_maps


_NC_CACHE = {}
LAST = {}


def kernel(x, mask, w_qkv, w_proj, b_proj, trace=False):
    from concourse.bass_utils import run_bass_kernel_spmd

    if "nc" not in _NC_CACHE:
        _NC_CACHE["nc"] = build_nc()
    nc = _NC_CACHE["nc"]
    in_maps = host_prep(x, mask, w_qkv, w_proj, b_proj)
    import tempfile

    tmpdir = tempfile.mkdtemp(prefix="bass_attn_")
    LAST["tmpdir"] = tmpdir
    res = run_bass_kernel_spmd(nc, in_maps, list(range(B)), trace=trace, tmpdir=tmpdir)
    LAST["exec_time_ns"] = res.exec_time_ns
    LAST["results"] = res
    out = np.stack([res.results[b]["y"] for b in range(B)], 0)
    return out.astype(np.float32)
